# revision 42
# baseline (speedup 1.0000x reference)
"""Multi-head attention (B=8, N=1024, C=768, H=12) on 8 TRN2 NeuronCores.

Sharding: data-parallel over batch - core i computes batch element i fully.
Weights / bias tables are replicated. No collectives.

Final design (all matmuls bf16, f32 PSUM accumulation):
  * Key compaction: host gathers valid key rows; padded rows carry ebias=0
    so they vanish from numerator and denominator.
  * S^T via 64-row PE tiling: the two heads of a pair sit on partition
    halves of qt/kt; their 64-contraction matmuls are issued back-to-back
    at tile positions (0,0)/(64,0) into adjacent PSUM banks and execute
    concurrently (~2x on the score matmuls).
  * Attention steps flatten (query-chunk ic x key-tile jt) per head pair,
    with PV lagged 3 steps behind S^T so exp (scalar) -> P=e*ebias (DVE)
    -> PV (PE) pipelines across the ic boundary; next pair's Q/K
    projection chains interleave to keep the PE dense (HAM stays warm).
  * PSUM: qk-proj(2) + st(2x2) + pv(2) = 8 banks.
  * PV lhsT = [V | ones]: softmax denominator lands in PSUM row 64 free;
    one row copy per (pair, ic) (alternating scalar/DVE) extracts it
    FIRST in the evacuation (it heads the longest recip chain); den DMAs
    issue from the GpSimd queue, recip broadcasts from sync, and the ou
    normalization multiplies are deferred+spread into the next pair's
    phases so no engine head-of-line blocks on the recip chain.
  * Output projection transposed (out^T[f, tok]); written bf16; the host
    casts back and adds b_proj.
  * Startup: wqk is split into per-pair tiles and xT into per-chunk tiles
    (whole-tile deps) so K0/Q0 unblock after ~2.5MB of DMA instead of
    ~4.5MB; priority order xkT, wv, wqk_p0, xT_c0, ebias(0,0), xT_c1,
    wqk_p1.., ebias(0,1); wp late. Compute: warm-up, V-proj, K0/Q0,
    pair loop.
"""

import numpy as np
import ml_dtypes

DIM = 768
NUM_HEADS = 12
HD = 64
N_TOK = 1024
B = 8
SCALE = HD ** -0.5

_BUILD_CACHE = {}


def _build_nc(N=N_TOK, H=NUM_HEADS, NVT=None, mmdt_name="bfloat16"):
    import concourse.bass as bass
    import concourse.mybir as mybir
    import concourse.tile as tile
    from concourse import bacc

    # Pin every activation to the one table set containing exp/ln/copy.
    if not getattr(bacc, "_act_tables_pinned", False):
        _orig_gat = bacc.get_activation_tables

        def _pinned_gat(arch):
            tabs = _orig_gat(arch)
            want = None
            for name, funcs in tabs.items():
                fn = {f.name.lower() for f in funcs}
                if "exp" in fn and "ln" in fn and "copy" in fn:
                    want = name
                    break
            if want is None:
                return tabs
            return {
                name: (funcs if name == want else set())
                for name, funcs in tabs.items()
            }

        bacc.get_activation_tables = _pinned_gat
        bacc._act_tables_pinned = True

    f32 = mybir.dt.float32
    mmdt = getattr(mybir.dt, mmdt_name)
    Exp = mybir.ActivationFunctionType.Exp
    Ln = mybir.ActivationFunctionType.Ln
    mult = mybir.AluOpType.mult
    add = mybir.AluOpType.add

    C = H * HD
    if NVT is None:
        NVT = N // 128
    NV = NVT * 128
    KO = C // 128
    HP = H // 2
    IC = N // 512
    NJG = (NVT + 1) // 2            # jt groups of 2
    kchunks = [(k0, min(512, NV - k0)) for k0 in range(0, NV, 512)]
    fchunks = [(f0, min(512, C - f0)) for f0 in range(0, C, 512)]

    nc = bacc.Bacc(None)
    xT_d = nc.declare_dram_parameter("xT", [C, N], mmdt, isOutput=False)
    xkT_d = nc.declare_dram_parameter("xkT", [C, NV], mmdt, isOutput=False)
    # pair-major columns: [Q_p0 | K_p0 | Q_p1 | K_p1 | ...] each 128 wide
    wqk_d = nc.declare_dram_parameter("wqk", [C, 2 * C], mmdt, isOutput=False)
    wv_d = nc.declare_dram_parameter("wv", [C, C], mmdt, isOutput=False)
    wp_d = nc.declare_dram_parameter("wp", [C, C], mmdt, isOutput=False)
    # rows (pair, ic, jtg, p); cols (jt01, h01, x)
    eb_d = nc.declare_dram_parameter(
        "ebias", [HP * IC * NJG * 128, 2 * 2 * 512], mmdt, isOutput=False
    )
    out_d = nc.declare_dram_parameter("out", [C, N], mmdt, isOutput=True)

    with tile.TileContext(nc) as tc:
        with (
            tc.tile_pool(name="singles", bufs=1) as singles,
            tc.tile_pool(name="dram", bufs=1, space="DRAM") as drampool,
        ):
            # ---- input loads in priority order ----
            KH = (KO + 1) // 2
            xkT_r = xkT_d.rearrange("(ko p) n -> p ko n", p=128)
            xka = singles.tile([128, KH, NV], mmdt)
            nc.sync.dma_start(xka[:], xkT_r[:, :KH])
            xkb = None
            if KH < KO:
                xkb = singles.tile([128, KO - KH, NV], mmdt)
                nc.sync.dma_start(xkb[:], xkT_r[:, KH:])
            wv = singles.tile([128, KO, C], mmdt)
            nc.sync.dma_start(
                wv[:], wv_d.rearrange("(ko p) m -> p ko m", p=128)
            )
            wqk_r = wqk_d.rearrange("(ko p) m -> p ko m", p=128)
            wqk_p = [
                singles.tile([128, KO, 256], mmdt, name=f"wqk_p{i}")
                for i in range(HP)
            ]
            nc.sync.dma_start(wqk_p[0][:], wqk_r[:, :, 0:256])
            xT_r = xT_d.rearrange("(ko p) n -> p ko n", p=128)
            xT_c = [
                singles.tile([128, KO, 512], mmdt, name=f"xT_c{i}")
                for i in range(IC)
            ]
            nc.sync.dma_start(xT_c[0][:], xT_r[:, :, 0:512])

            def xkTs(ko):
                return xka[:, ko] if ko < KH else xkb[:, ko - KH]

            wp = singles.tile([128, KO, C], mmdt)

            vsb = singles.tile([128, NVT, H, HD + 1], mmdt)
            qt = singles.tile([128, HP, N], mmdt)
            kt = singles.tile([128, HP, NV], mmdt)
            ou = singles.tile([128, KO, N], mmdt)      # unnormalized O^T
            rb = singles.tile([128, HP, N], mmdt)      # broadcast recips
            den = singles.tile([128, N], f32)
            rden = singles.tile([128, N], f32)
            rdb = singles.tile([128, N], mmdt)
            drow = singles.tile([128, 4096], f32)      # den staging (row 64)
            rscratch = drampool.tile([H, N], mmdt)

            # ones column for the denominator (pad rows die via P=0)
            nc.vector.memset(vsb[:, :, :, HD : HD + 1], 1.0)

            eb_r = eb_d.rearrange(
                "(pr ic jtg p) x -> p pr ic jtg x", p=128, ic=IC, jtg=NJG
            )

            with (
                tc.tile_pool(name="qkv_psum", bufs=2, space="PSUM") as qp,
                tc.tile_pool(name="eb_pool", bufs=4) as eb_pool,
                tc.tile_pool(name="st_psum", bufs=2, space="PSUM") as st_psum,
                tc.tile_pool(name="pv_psum", bufs=1, space="PSUM") as pv_psum,
                tc.tile_pool(name="e_pool", bufs=3) as e_pool,
                tc.tile_pool(name="p_pool", bufs=5) as p_pool,
            ):
                if HP >= 5:
                    NORM_GROUPS = {
                        2: (0, 0, 6),
                        HP - 2: (6, 32, 2 * HP - 8),
                        HP - 1: (2 * HP - 2, 64, 2),
                    }
                else:
                    NORM_GROUPS = {HP - 1: (0, 0, H)}
                DEN_ROW = {}
                for _g0, _r0, _ng in NORM_GROUPS.values():
                    for _h in range(_g0, _g0 + _ng):
                        DEN_ROW[_h] = _r0 + _h - _g0

                pending_norms = []

                def _normalize_group(g0, r0, ng, ic):
                    i0 = 512 * ic
                    nc.scalar.activation(
                        rden[r0 : r0 + ng, i0 : i0 + 512],
                        den[r0 : r0 + ng, i0 : i0 + 512],
                        Ln,
                    )
                    nc.scalar.activation(
                        rdb[r0 : r0 + ng, i0 : i0 + 512],
                        rden[r0 : r0 + ng, i0 : i0 + 512],
                        Exp,
                        scale=-1.0,
                    )
                    nc.sync.dma_start(
                        rscratch[g0 : g0 + ng, i0 : i0 + 512],
                        rdb[r0 : r0 + ng, i0 : i0 + 512],
                    )
                    for h in range(g0, g0 + ng):
                        ho = 64 * (h % 2)
                        nc.sync.dma_start(
                            rb[ho : ho + 64, h // 2, i0 : i0 + 512],
                            bass.AP(
                                tensor=rscratch.tensor,
                                offset=rscratch[h, i0].offset,
                                ap=[[0, 64], [1, 512]],
                            ),
                        )
                    # the ou multiplies are deferred to the next pair (or
                    # the tail) so the DVE never head-of-line blocks on the
                    # recip-broadcast DMA chain, and GpSimd (which shares the
                    # DVE SBUF port) never runs tensor ops during hot compute
                    pending_norms.append((g0, ng, ic))

                norm_tts = []

                def _apply_norms(limit=None):
                    while pending_norms:
                        g0, ng, ic = pending_norms.pop(0)
                        i0 = 512 * ic
                        for sl in range(g0 // 2, (g0 + ng) // 2):
                            norm_tts.append((sl, i0))
                    n = 0
                    while norm_tts and (limit is None or n < limit):
                        sl, i0 = norm_tts.pop(0)
                        nc.vector.tensor_tensor(
                            ou[:, sl, i0 : i0 + 512],
                            ou[:, sl, i0 : i0 + 512],
                            rb[:, sl, i0 : i0 + 512],
                            mult,
                        )
                        n += 1

                # PE warm-up while input DMAs stream
                warm = e_pool.tile([128, 512], mmdt, tag="e", name="warm_in")
                nc.vector.memset(warm[:], 0.0)
                # trigger the lazy ACT_TABLE_LOAD (~2.7us) now, not at the
                # first V-projection evacuation
                nc.vector.memset(drow[0:1, 0:8], 1.0)
                nc.scalar.activation(rden[0:1, 0:8], drow[0:1, 0:8], Exp)
                wps = qp.tile([128, 512], f32, tag="ps", name="warm_ps")
                for _w in range(24):
                    nc.tensor.matmul(
                        wps[:],
                        lhsT=warm[:, :128],
                        rhs=warm[:],
                        start=True,
                        stop=True,
                    )

                # ---------- Q/K projection helper (K first: xkT+wqk arrive
                # ---------- before xT) ----------
                def qk_chunks(pair):
                    jobs = []
                    for mt, dst, chunks in (
                        (2 * pair + 1, kt, kchunks),
                        (2 * pair, qt, [(i0, 512) for i0 in range(0, N, 512)]),
                    ):
                        for i0, il in chunks:
                            def run(mt=mt, dst=dst, i0=i0, il=il, pair=pair):
                                ps = qp.tile([128, 512], f32, tag="ps")
                                co = 128 * (mt % 2)
                                for ko in range(KO):
                                    rhs = (
                                        xT_c[i0 // 512][:, ko, :il]
                                        if mt % 2 == 0
                                        else xkTs(ko)[:, i0 : i0 + il]
                                    )
                                    nc.tensor.matmul(
                                        ps[:, :il],
                                        lhsT=wqk_p[pair][:, ko, co : co + 128],
                                        rhs=rhs,
                                        start=(ko == 0),
                                        stop=(ko == KO - 1),
                                    )
                                nc.vector.tensor_copy(
                                    dst[:, pair, i0 : i0 + il], ps[:, :il]
                                )
                            jobs.append(run)
                    return jobs

                eb_tiles = {}

                def load_eb(pair, ic):
                    ebt = eb_pool.tile(
                        [128, NJG, 2048], mmdt, tag="eb",
                        name=f"eb_{pair}_{ic}",
                    )
                    nc.sync.dma_start(ebt[:], eb_r[:, pair, ic])
                    eb_tiles[(pair, ic)] = ebt

                # ---------- V projection (evacuated on scalar) ----------
                for jt in range(NVT):
                    for f0, fl in fchunks:
                        ps = qp.tile([128, 512], f32, tag="ps")
                        for ko in range(KO):
                            nc.tensor.matmul(
                                ps[:, :fl],
                                lhsT=xkTs(ko)[:, 128 * jt : 128 * jt + 128],
                                rhs=wv[:, ko, f0 : f0 + fl],
                                start=(ko == 0),
                                stop=(ko == KO - 1),
                            )
                        h0, nh = f0 // HD, fl // HD
                        nc.scalar.copy(
                            vsb[:, jt, h0 : h0 + nh, 0:HD],
                            ps[:, :fl].rearrange("p (h d) -> p h d", d=HD),
                        )

                load_eb(0, 0)
                nc.sync.dma_start(xT_c[1][:], xT_r[:, :, 512:N])
                for p_ in range(1, HP):
                    nc.sync.dma_start(
                        wqk_p[p_][:], wqk_r[:, :, 256 * p_ : 256 * p_ + 256]
                    )
                # pair 0's K+Q upfront (emitted after every input dma_start
                # above so writer-before-reader program order holds)
                for job in qk_chunks(0):
                    job()
                load_eb(0, 1)

                # late weight load (projection only)
                nc.sync.dma_start(
                    wp[:], wp_d.rearrange("(ko p) m -> p ko m", p=128)
                )

                # ---------- attention pair loop ----------
                # steps flatten (ic, jt); S^T for step k, PV for step k-2,
                # so the drain of ic0 overlaps the warm-up of ic1.
                LAGS = 3
                for pair in range(HP):
                    if pair + 1 < HP:
                        load_eb(pair + 1, 0)
                        load_eb(pair + 1, 1)
                    next_jobs = qk_chunks(pair + 1) if pair + 1 < HP else []
                    jidx = 0
                    h0, h1 = 2 * pair, 2 * pair + 1
                    steps = [(ic, jt) for ic in range(IC) for jt in range(NVT)]
                    pv_tiles = {}
                    pts = {}

                    def evac(ic, pv2, pair=pair, h0=h0, h1=h1):
                        i0 = 512 * ic
                        slot = 1024 * ((pair * IC + ic) % 4)
                        if (pair * IC + ic) % 2 == 0:
                            nc.scalar.copy(
                                drow[64:65, slot : slot + 1024],
                                pv2[HD : HD + 1, :],
                            )
                        else:
                            nc.vector.tensor_copy(
                                drow[64:65, slot : slot + 1024],
                                pv2[HD : HD + 1, :],
                            )
                        r0, r1 = DEN_ROW[h0], DEN_ROW[h1]
                        nc.gpsimd.dma_start(
                            den[r0 : r0 + 1, i0 : i0 + 512],
                            drow[64:65, slot : slot + 512],
                        )
                        nc.gpsimd.dma_start(
                            den[r1 : r1 + 1, i0 : i0 + 512],
                            drow[64:65, slot + 512 : slot + 1024],
                        )
                        if pair in NORM_GROUPS:
                            g0, r0g, ng = NORM_GROUPS[pair]
                            _normalize_group(g0, r0g, ng, ic)
                        nc.vector.tensor_copy(
                            ou[0:64, pair, i0 : i0 + 512], pv2[0:HD, 0:512]
                        )
                        nc.vector.tensor_copy(
                            ou[64:128, pair, i0 : i0 + 512],
                            pv2[0:HD, 512:1024],
                        )

                    for ph in range(len(steps) + LAGS):
                        # PV for step ph-LAGS (full 128 mode)
                        if ph >= LAGS and ph - LAGS < len(steps):
                            ic, jd = steps[ph - LAGS]
                            if jd == 0:
                                pv_tiles[ic] = pv_psum.tile(
                                    [128, 1024], f32, tag="pv",
                                    name=f"pv_{pair}_{ic}",
                                )
                            pv2 = pv_tiles[ic]
                            pd = pts.pop((ic, jd))
                            for hh in (0, 1):
                                nc.tensor.matmul(
                                    pv2[: HD + 1, 512 * hh : 512 * hh + 512],
                                    lhsT=vsb[:, jd, h0 + hh, :],
                                    rhs=pd[:, 512 * hh : 512 * hh + 512],
                                    start=(jd == 0),
                                    stop=(jd == NVT - 1),
                                )
                            if jd == NVT - 1:
                                evac(ic, pv_tiles.pop(ic))
                        # one Q/K chain of the next pair; once a pair's
                        # chains run out (and for the last pair, always),
                        # feed dummy matmuls so HAM never re-throttles
                        if ph % 2 == 1:
                            if jidx < len(next_jobs):
                                next_jobs[jidx]()
                                jidx += 1
                            elif ph <= 11:
                                dps = qp.tile([128, 512], f32, tag="ps")
                                for _d in range(3):
                                    nc.tensor.matmul(
                                        dps[:],
                                        lhsT=wqk_p[0][:, 0, 0:128],
                                        rhs=wqk_p[0][:, :2, :].rearrange(
                                            "p a b -> p (a b)"
                                        ),
                                        start=True,
                                        stop=True,
                                    )
                        # one deferred normalize multiply per phase, once its
                        # recip-broadcast chain has had ~4 phases to land
                        if ph >= 4:
                            _apply_norms(limit=1)
                        # S^T for step ph (64-row tiled, concurrent heads)
                        if ph < len(steps):
                            ic, jt = steps[ph]
                            i0 = 512 * ic
                            ebt = eb_tiles[(pair, ic)]
                            st2 = st_psum.tile(
                                [128, 1024], f32, tag="st",
                                name=f"st_{pair}_{ic}_{jt}",
                            )
                            for hh, ho in ((0, 0), (1, 64)):
                                nc.tensor.matmul(
                                    st2[:, 512 * hh : 512 * hh + 512],
                                    lhsT=kt[
                                        ho : ho + 64, pair,
                                        128 * jt : 128 * jt + 128,
                                    ],
                                    rhs=qt[ho : ho + 64, pair, i0 : i0 + 512],
                                    start=True,
                                    stop=True,
                                )
                            e2 = e_pool.tile([128, 1024], mmdt, tag="e")
                            nc.scalar.activation(e2[:], st2[:], Exp)
                            p2 = p_pool.tile([128, 1024], mmdt, tag="p")
                            nc.vector.tensor_tensor(
                                p2[:],
                                e2[:],
                                ebt[:, jt // 2, 1024 * (jt % 2) :
                                    1024 * (jt % 2) + 1024],
                                mult,
                            )
                            pts[(ic, jt)] = p2
                    for ic in range(IC):
                        eb_tiles.pop((pair, ic))
                    while jidx < len(next_jobs):
                        next_jobs[jidx]()
                        jidx += 1
                    if pair == HP - 1:
                        _apply_norms()

            # ---------------- output projection (transposed) ----------------
            with (
                tc.tile_pool(name="proj_psum", bufs=7, space="PSUM") as proj_psum,
                tc.tile_pool(name="o_pool", bufs=4) as o_pool,
            ):
                groups = [
                    (ft, t0)
                    for ft in range(KO)
                    for t0 in range(0, N, 512)
                ]
                LAG = min(6, len(groups) - 1) if KO > 1 else 0
                psums = {}
                for g in range(len(groups) + LAG):
                    if g < len(groups):
                        ft, t0 = groups[g]
                        ps = proj_psum.tile(
                            [128, 512], f32, tag="ps", name=f"pps_{g}"
                        )
                        for ko in range(KO - 1):
                            nc.tensor.matmul(
                                ps[:],
                                lhsT=wp[:, ko, 128 * ft : 128 * ft + 128],
                                rhs=ou[:, ko, t0 : t0 + 512],
                                start=(ko == 0),
                                stop=False,
                            )
                        psums[g] = ps
                    if g >= LAG:
                        gd = g - LAG
                        ft, t0 = groups[gd]
                        ps = psums.pop(gd)
                        nc.tensor.matmul(
                            ps[:],
                            lhsT=wp[:, KO - 1, 128 * ft : 128 * ft + 128],
                            rhs=ou[:, KO - 1, t0 : t0 + 512],
                            start=(KO == 1),
                            stop=True,
                        )
                        if gd % 2 == 0:
                            ot2 = o_pool.tile(
                                [128, 1024], mmdt, tag="ot", name=f"ot_{gd}"
                            )
                            nc.vector.tensor_copy(ot2[:, 0:512], ps[:])
                        else:
                            nc.scalar.copy(ot2[:, 512:1024], ps[:])
                            # one double-width DMA per ft row; alternate the
                            # issue queue so the tail issues in parallel
                            eng = nc.sync if ft % 2 == 0 else nc.gpsimd
                            eng.dma_start(
                                out_d[128 * ft : 128 * ft + 128, :], ot2[:]
                            )

    nc.finalize()
    return nc


def _host_pack(x, w_qkv, w_proj, b_proj, bias_table, key_padding_mask,
               N=N_TOK, H=NUM_HEADS, mmdt_name="bfloat16"):
    """Host-side layout: per-core input dicts (core i <- batch i)."""
    np_mmdt = ml_dtypes.bfloat16 if mmdt_name == "bfloat16" else np.float32
    C = H * HD
    HP = H // 2
    IC = N // 512

    x = np.asarray(x, np.float32)
    mask = np.asarray(key_padding_mask).astype(bool)
    Bb = x.shape[0]

    valid = [np.where(mask[b])[0] for b in range(Bb)]
    nv_max = max(1, max(len(v) for v in valid))
    NVT = (nv_max + 127) // 128
    NV = NVT * 128
    NJG = (NVT + 1) // 2

    w_qkv = np.asarray(w_qkv, np.float32)
    wqkT = np.ascontiguousarray(w_qkv[: 2 * C].T).astype(np.float32)
    wqkT[:, :C] *= SCALE                      # fold softmax scale into W_q
    blocks = []
    for p in range(HP):
        blocks.append(wqkT[:, 128 * p : 128 * p + 128])
        blocks.append(wqkT[:, C + 128 * p : C + 128 * p + 128])
    wqk = np.ascontiguousarray(np.concatenate(blocks, axis=1)).astype(np_mmdt)
    wv = np.ascontiguousarray(w_qkv[2 * C :].T).astype(np_mmdt)
    wp = np.ascontiguousarray(np.asarray(w_proj, np.float32).T).astype(np_mmdt)
    bp = np.asarray(b_proj, np.float32)

    etab = np.exp(np.asarray(bias_table, np.float32))   # [2N-1, H]
    iota = np.arange(N)

    in_maps = []
    for b in range(Bb):
        v = valid[b]
        nv = len(v)
        xT = np.ascontiguousarray(x[b].T).astype(np_mmdt)
        xk = np.zeros((NV, C), np.float32)
        xk[:nv] = x[b][v]
        xkT = np.ascontiguousarray(xk.T).astype(np_mmdt)
        idx = np.zeros((NV, N), np.int32)
        idx[:nv] = v[:, None] - iota[None, :] + N - 1
        eb = etab[idx, :]                     # [NV, N, H]
        eb[nv:] = 0.0
        # pad jt tiles to full groups of 2
        ebp = np.zeros((NJG * 256, N, H), np.float32)
        ebp[:NV] = eb
        # [jtg, jt01, p, ic, x, h]
        ebp = ebp.reshape(NJG, 2, 128, IC, 512, H)
        # -> [h, ic, jtg, p, jt01, x]
        ebp = ebp.transpose(5, 3, 0, 2, 1, 4)
        # -> [pair, h01, ic, jtg, p, jt01, x]
        ebp = ebp.reshape(HP, 2, IC, NJG, 128, 2, 512)
        # -> [pair, ic, jtg, p, jt01, h01, x]
        ebp = ebp.transpose(0, 2, 3, 4, 5, 1, 6)
        ebias = np.ascontiguousarray(
            ebp.reshape(HP * IC * NJG * 128, 2 * 2 * 512)
        ).astype(np_mmdt)
        in_maps.append({
            "xT": xT, "xkT": xkT, "wqk": wqk, "wv": wv, "wp": wp,
            "ebias": ebias,
        })
    return in_maps, NVT, bp


def _run(x, w_qkv, w_proj, b_proj, bias_table, key_padding_mask, trace=False):
    from concourse.bass_utils import run_bass_kernel_spmd

    in_maps, NVT, bp = _host_pack(
        x, w_qkv, w_proj, b_proj, bias_table, key_padding_mask
    )
    key = ("v4", N_TOK, NUM_HEADS, NVT)
    if key not in _BUILD_CACHE:
        _BUILD_CACHE[key] = _build_nc(NVT=NVT)
    nc = _BUILD_CACHE[key]
    res = run_bass_kernel_spmd(nc, in_maps, core_ids=list(range(B)), trace=trace)
    out = np.stack(
        [np.asarray(res.results[i]["out"]).T for i in range(B)]
    )
    return out.astype(np.float32) + bp[None, None, :], res


def kernel(x, w_qkv, w_proj, b_proj, bias_table, key_padding_mask):
    # The Bass/tile build is sensitive to Python hash randomization: with an
    # unseeded interpreter the emitted schedule is bimodal (~162-168us good
    # mode vs ~185-195us bad mode on identical source). Re-invoke ourselves
    # in a PYTHONHASHSEED=0 subprocess for the deterministic good mode; fall
    # back to an in-process run on any failure (e.g. device already held).
    import os
    import sys
    import subprocess
    import tempfile

    if os.environ.get("PYTHONHASHSEED") != "0":
        try:
            with tempfile.TemporaryDirectory() as td:
                inp = os.path.join(td, "in.npz")
                outp = os.path.join(td, "out.npy")
                np.savez(
                    inp, x=np.asarray(x), w_qkv=np.asarray(w_qkv),
                    w_proj=np.asarray(w_proj), b_proj=np.asarray(b_proj),
                    bias_table=np.asarray(bias_table),
                    key_padding_mask=np.asarray(key_padding_mask),
                )
                env = dict(os.environ, PYTHONHASHSEED="0")
                r = subprocess.run(
                    [sys.executable, os.path.abspath(__file__), inp, outp],
                    env=env, timeout=1500,
                )
                if r.returncode == 0 and os.path.exists(outp):
                    return np.load(outp)
        except Exception:
            pass
    out, _ = _run(x, w_qkv, w_proj, b_proj, bias_table, key_padding_mask)
    return out


if __name__ == "__main__":
    import sys as _sys
    _inp, _outp = _sys.argv[1], _sys.argv[2]
    _d = np.load(_inp)
    _out, _ = _run(**{k: _d[k] for k in _d.files})
    np.save(_outp, _out)


# revision 43
# speedup vs baseline: 1.1638x; 1.1638x over previous
"""Multi-head attention (B=8, N=1024, C=768, H=12) on 8 TRN2 NeuronCores.

Sharding: data-parallel over batch - core i computes batch element i fully.
Weights / bias tables are replicated. No collectives.

Final design (all matmuls bf16, f32 PSUM accumulation):
  * Key compaction: host gathers valid key rows; padded rows carry ebias=0
    so they vanish from numerator and denominator.
  * S^T via 64-row PE tiling: the two heads of a pair sit on partition
    halves of qt/kt; their 64-contraction matmuls are issued back-to-back
    at tile positions (0,0)/(64,0) into adjacent PSUM banks and execute
    concurrently (~2x on the score matmuls).
  * Attention steps flatten (query-chunk ic x key-tile jt) per head pair,
    with PV lagged 3 steps behind S^T so exp (scalar) -> P=e*ebias (DVE)
    -> PV (PE) pipelines across the ic boundary; next pair's Q/K
    projection chains interleave to keep the PE dense (HAM stays warm).
  * PSUM: qk-proj(2) + st(2x2) + pv(2) = 8 banks.
  * PV lhsT = [V | ones]: softmax denominator lands in PSUM row 64 free;
    one row copy per (pair, ic) (alternating scalar/DVE) extracts it
    FIRST in the evacuation (it heads the longest recip chain); den DMAs
    issue from the GpSimd queue, recip broadcasts from sync, and the ou
    normalization multiplies are deferred+spread into the next pair's
    phases so no engine head-of-line blocks on the recip chain.
  * Output projection transposed (out^T[f, tok]); written bf16; the host
    casts back and adds b_proj.
  * Startup: wqk is split into per-pair tiles and xT into per-chunk tiles
    (whole-tile deps) so K0/Q0 unblock after ~2.5MB of DMA instead of
    ~4.5MB; priority order xkT, wv, wqk_p0, xT_c0, ebias(0,0), xT_c1,
    wqk_p1.., ebias(0,1); wp late. Compute: warm-up, V-proj, K0/Q0,
    pair loop.
"""

import numpy as np
import ml_dtypes

DIM = 768
NUM_HEADS = 12
HD = 64
N_TOK = 1024
B = 8
SCALE = HD ** -0.5

_BUILD_CACHE = {}


def _build_nc(N=N_TOK, H=NUM_HEADS, NVT=None, mmdt_name="bfloat16"):
    import concourse.bass as bass
    import concourse.mybir as mybir
    import concourse.tile as tile
    from concourse import bacc

    # Pin every activation to the one table set containing exp/ln/copy.
    if not getattr(bacc, "_act_tables_pinned", False):
        _orig_gat = bacc.get_activation_tables

        def _pinned_gat(arch):
            tabs = _orig_gat(arch)
            want = None
            for name, funcs in tabs.items():
                fn = {f.name.lower() for f in funcs}
                if "exp" in fn and "ln" in fn and "copy" in fn:
                    want = name
                    break
            if want is None:
                return tabs
            return {
                name: (funcs if name == want else set())
                for name, funcs in tabs.items()
            }

        bacc.get_activation_tables = _pinned_gat
        bacc._act_tables_pinned = True

    f32 = mybir.dt.float32
    mmdt = getattr(mybir.dt, mmdt_name)
    Exp = mybir.ActivationFunctionType.Exp
    Ln = mybir.ActivationFunctionType.Ln
    mult = mybir.AluOpType.mult
    add = mybir.AluOpType.add

    C = H * HD
    if NVT is None:
        NVT = N // 128
    NV = NVT * 128
    KO = C // 128
    HP = H // 2
    IC = N // 512
    NJG = (NVT + 1) // 2            # jt groups of 2
    kchunks = [(k0, min(512, NV - k0)) for k0 in range(0, NV, 512)]
    fchunks = [(f0, min(512, C - f0)) for f0 in range(0, C, 512)]

    nc = bacc.Bacc(None)
    xT_d = nc.declare_dram_parameter("xT", [C, N], mmdt, isOutput=False)
    xkT_d = nc.declare_dram_parameter("xkT", [C, NV], mmdt, isOutput=False)
    # pair-major columns: [Q_p0 | K_p0 | Q_p1 | K_p1 | ...] each 128 wide
    wqk_d = nc.declare_dram_parameter("wqk", [C, 2 * C], mmdt, isOutput=False)
    wv_d = nc.declare_dram_parameter("wv", [C, C], mmdt, isOutput=False)
    wp_d = nc.declare_dram_parameter("wp", [C, C], mmdt, isOutput=False)
    # rows (pair, ic, jtg, p); cols (jt01, h01, x)
    eb_d = nc.declare_dram_parameter(
        "ebias", [HP * IC * NJG * 128, 2 * 2 * 512], mmdt, isOutput=False
    )
    out_d = nc.declare_dram_parameter("out", [C, N], mmdt, isOutput=True)

    with tile.TileContext(nc) as tc:
        with (
            tc.tile_pool(name="singles", bufs=1) as singles,
            tc.tile_pool(name="dram", bufs=1, space="DRAM") as drampool,
        ):
            # ---- input loads in priority order ----
            KH = (KO + 1) // 2
            xkT_r = xkT_d.rearrange("(ko p) n -> p ko n", p=128)
            xka = singles.tile([128, KH, NV], mmdt)
            nc.sync.dma_start(xka[:], xkT_r[:, :KH])
            xkb = None
            if KH < KO:
                xkb = singles.tile([128, KO - KH, NV], mmdt)
                nc.sync.dma_start(xkb[:], xkT_r[:, KH:])
            wv = singles.tile([128, KO, C], mmdt)
            nc.sync.dma_start(
                wv[:], wv_d.rearrange("(ko p) m -> p ko m", p=128)
            )
            wqk_r = wqk_d.rearrange("(ko p) m -> p ko m", p=128)
            wqk_p = [
                singles.tile([128, KO, 256], mmdt, name=f"wqk_p{i}")
                for i in range(HP)
            ]
            nc.sync.dma_start(wqk_p[0][:], wqk_r[:, :, 0:256])
            xT_r = xT_d.rearrange("(ko p) n -> p ko n", p=128)
            xT_c = [
                singles.tile([128, KO, 512], mmdt, name=f"xT_c{i}")
                for i in range(IC)
            ]
            nc.sync.dma_start(xT_c[0][:], xT_r[:, :, 0:512])

            def xkTs(ko):
                return xka[:, ko] if ko < KH else xkb[:, ko - KH]

            wp = singles.tile([128, KO, C], mmdt)

            vsb = singles.tile([128, NVT, H, HD + 1], mmdt)
            qt = singles.tile([128, HP, N], mmdt)
            kt = singles.tile([128, HP, NV], mmdt)
            ou = singles.tile([128, KO, N], mmdt)      # unnormalized O^T
            rb = singles.tile([128, HP, N], mmdt)      # broadcast recips
            den = singles.tile([128, N], f32)
            rden = singles.tile([128, N], f32)
            rdb = singles.tile([128, N], mmdt)
            drow = singles.tile([128, 4096], f32)      # den staging (row 64)
            rscratch = drampool.tile([H, N], mmdt)

            # ones column for the denominator (pad rows die via P=0)
            nc.vector.memset(vsb[:, :, :, HD : HD + 1], 1.0)

            eb_r = eb_d.rearrange(
                "(pr ic jtg p) x -> p pr ic jtg x", p=128, ic=IC, jtg=NJG
            )

            with (
                tc.tile_pool(name="qkv_psum", bufs=2, space="PSUM") as qp,
                tc.tile_pool(name="eb_pool", bufs=4) as eb_pool,
                tc.tile_pool(name="st_psum", bufs=2, space="PSUM") as st_psum,
                tc.tile_pool(name="pv_psum", bufs=1, space="PSUM") as pv_psum,
                tc.tile_pool(name="e_pool", bufs=3) as e_pool,
                tc.tile_pool(name="p_pool", bufs=5) as p_pool,
            ):
                if HP >= 5:
                    NORM_GROUPS = {
                        2: (0, 0, 6),
                        HP - 2: (6, 32, 2 * HP - 8),
                        HP - 1: (2 * HP - 2, 64, 2),
                    }
                else:
                    NORM_GROUPS = {HP - 1: (0, 0, H)}
                DEN_ROW = {}
                for _g0, _r0, _ng in NORM_GROUPS.values():
                    for _h in range(_g0, _g0 + _ng):
                        DEN_ROW[_h] = _r0 + _h - _g0

                pending_norms = []

                def _normalize_group(g0, r0, ng, ic):
                    i0 = 512 * ic
                    nc.scalar.activation(
                        rden[r0 : r0 + ng, i0 : i0 + 512],
                        den[r0 : r0 + ng, i0 : i0 + 512],
                        Ln,
                    )
                    nc.scalar.activation(
                        rdb[r0 : r0 + ng, i0 : i0 + 512],
                        rden[r0 : r0 + ng, i0 : i0 + 512],
                        Exp,
                        scale=-1.0,
                    )
                    nc.sync.dma_start(
                        rscratch[g0 : g0 + ng, i0 : i0 + 512],
                        rdb[r0 : r0 + ng, i0 : i0 + 512],
                    )
                    for h in range(g0, g0 + ng):
                        ho = 64 * (h % 2)
                        nc.sync.dma_start(
                            rb[ho : ho + 64, h // 2, i0 : i0 + 512],
                            bass.AP(
                                tensor=rscratch.tensor,
                                offset=rscratch[h, i0].offset,
                                ap=[[0, 64], [1, 512]],
                            ),
                        )
                    # the ou multiplies are deferred to the next pair (or
                    # the tail) so the DVE never head-of-line blocks on the
                    # recip-broadcast DMA chain, and GpSimd (which shares the
                    # DVE SBUF port) never runs tensor ops during hot compute
                    pending_norms.append((g0, ng, ic))

                norm_tts = []

                def _apply_norms(limit=None):
                    while pending_norms:
                        g0, ng, ic = pending_norms.pop(0)
                        i0 = 512 * ic
                        for sl in range(g0 // 2, (g0 + ng) // 2):
                            norm_tts.append((sl, i0))
                    n = 0
                    while norm_tts and (limit is None or n < limit):
                        sl, i0 = norm_tts.pop(0)
                        nc.vector.tensor_tensor(
                            ou[:, sl, i0 : i0 + 512],
                            ou[:, sl, i0 : i0 + 512],
                            rb[:, sl, i0 : i0 + 512],
                            mult,
                        )
                        n += 1

                # PE warm-up while input DMAs stream
                warm = e_pool.tile([128, 512], mmdt, tag="e", name="warm_in")
                nc.vector.memset(warm[:], 0.0)
                # trigger the lazy ACT_TABLE_LOAD (~2.7us) now, not at the
                # first V-projection evacuation
                nc.vector.memset(drow[0:1, 0:8], 1.0)
                nc.scalar.activation(rden[0:1, 0:8], drow[0:1, 0:8], Exp)
                wps = qp.tile([128, 512], f32, tag="ps", name="warm_ps")
                for _w in range(24):
                    nc.tensor.matmul(
                        wps[:],
                        lhsT=warm[:, :128],
                        rhs=warm[:],
                        start=True,
                        stop=True,
                    )

                # ---------- Q/K projection helper (K first: xkT+wqk arrive
                # ---------- before xT) ----------
                def qk_chunks(pair):
                    jobs = []
                    for mt, dst, chunks in (
                        (2 * pair + 1, kt, kchunks),
                        (2 * pair, qt, [(i0, 512) for i0 in range(0, N, 512)]),
                    ):
                        for i0, il in chunks:
                            def run(mt=mt, dst=dst, i0=i0, il=il, pair=pair):
                                ps = qp.tile([128, 512], f32, tag="ps")
                                co = 128 * (mt % 2)
                                for ko in range(KO):
                                    rhs = (
                                        xT_c[i0 // 512][:, ko, :il]
                                        if mt % 2 == 0
                                        else xkTs(ko)[:, i0 : i0 + il]
                                    )
                                    nc.tensor.matmul(
                                        ps[:, :il],
                                        lhsT=wqk_p[pair][:, ko, co : co + 128],
                                        rhs=rhs,
                                        start=(ko == 0),
                                        stop=(ko == KO - 1),
                                    )
                                nc.vector.tensor_copy(
                                    dst[:, pair, i0 : i0 + il], ps[:, :il]
                                )
                            jobs.append(run)
                    return jobs

                eb_tiles = {}

                def load_eb(pair, ic):
                    ebt = eb_pool.tile(
                        [128, NJG, 2048], mmdt, tag="eb",
                        name=f"eb_{pair}_{ic}",
                    )
                    nc.sync.dma_start(ebt[:], eb_r[:, pair, ic])
                    eb_tiles[(pair, ic)] = ebt

                # ---------- V projection (evacuated on scalar) ----------
                for jt in range(NVT):
                    for f0, fl in fchunks:
                        ps = qp.tile([128, 512], f32, tag="ps")
                        for ko in range(KO):
                            nc.tensor.matmul(
                                ps[:, :fl],
                                lhsT=xkTs(ko)[:, 128 * jt : 128 * jt + 128],
                                rhs=wv[:, ko, f0 : f0 + fl],
                                start=(ko == 0),
                                stop=(ko == KO - 1),
                            )
                        h0, nh = f0 // HD, fl // HD
                        nc.scalar.copy(
                            vsb[:, jt, h0 : h0 + nh, 0:HD],
                            ps[:, :fl].rearrange("p (h d) -> p h d", d=HD),
                        )

                load_eb(0, 0)
                nc.sync.dma_start(xT_c[1][:], xT_r[:, :, 512:N])
                for p_ in range(1, HP):
                    nc.sync.dma_start(
                        wqk_p[p_][:], wqk_r[:, :, 256 * p_ : 256 * p_ + 256]
                    )
                # pair 0's K+Q upfront (emitted after every input dma_start
                # above so writer-before-reader program order holds)
                for job in qk_chunks(0):
                    job()
                load_eb(0, 1)

                # late weight load (projection only)
                nc.sync.dma_start(
                    wp[:], wp_d.rearrange("(ko p) m -> p ko m", p=128)
                )

                # ---------- attention pair loop ----------
                # steps flatten (ic, jt); S^T for step k, PV for step k-2,
                # so the drain of ic0 overlaps the warm-up of ic1.
                LAGS = 3
                for pair in range(HP):
                    if pair + 1 < HP:
                        load_eb(pair + 1, 0)
                        load_eb(pair + 1, 1)
                    next_jobs = qk_chunks(pair + 1) if pair + 1 < HP else []
                    jidx = 0
                    h0, h1 = 2 * pair, 2 * pair + 1
                    steps = [(ic, jt) for ic in range(IC) for jt in range(NVT)]
                    pv_tiles = {}
                    pts = {}

                    def evac(ic, pv2, pair=pair, h0=h0, h1=h1):
                        i0 = 512 * ic
                        slot = 1024 * ((pair * IC + ic) % 4)
                        if (pair * IC + ic) % 2 == 0:
                            nc.scalar.copy(
                                drow[64:65, slot : slot + 1024],
                                pv2[HD : HD + 1, :],
                            )
                        else:
                            nc.vector.tensor_copy(
                                drow[64:65, slot : slot + 1024],
                                pv2[HD : HD + 1, :],
                            )
                        r0, r1 = DEN_ROW[h0], DEN_ROW[h1]
                        nc.gpsimd.dma_start(
                            den[r0 : r0 + 1, i0 : i0 + 512],
                            drow[64:65, slot : slot + 512],
                        )
                        nc.gpsimd.dma_start(
                            den[r1 : r1 + 1, i0 : i0 + 512],
                            drow[64:65, slot + 512 : slot + 1024],
                        )
                        if pair in NORM_GROUPS:
                            g0, r0g, ng = NORM_GROUPS[pair]
                            _normalize_group(g0, r0g, ng, ic)
                        nc.vector.tensor_copy(
                            ou[0:64, pair, i0 : i0 + 512], pv2[0:HD, 0:512]
                        )
                        nc.vector.tensor_copy(
                            ou[64:128, pair, i0 : i0 + 512],
                            pv2[0:HD, 512:1024],
                        )

                    for ph in range(len(steps) + LAGS):
                        # PV for step ph-LAGS (full 128 mode)
                        if ph >= LAGS and ph - LAGS < len(steps):
                            ic, jd = steps[ph - LAGS]
                            if jd == 0:
                                pv_tiles[ic] = pv_psum.tile(
                                    [128, 1024], f32, tag="pv",
                                    name=f"pv_{pair}_{ic}",
                                )
                            pv2 = pv_tiles[ic]
                            pd = pts.pop((ic, jd))
                            for hh in (0, 1):
                                nc.tensor.matmul(
                                    pv2[: HD + 1, 512 * hh : 512 * hh + 512],
                                    lhsT=vsb[:, jd, h0 + hh, :],
                                    rhs=pd[:, 512 * hh : 512 * hh + 512],
                                    start=(jd == 0),
                                    stop=(jd == NVT - 1),
                                )
                            if jd == NVT - 1:
                                evac(ic, pv_tiles.pop(ic))
                        # one Q/K chain of the next pair; once a pair's
                        # chains run out (and for the last pair, always),
                        # feed dummy matmuls so HAM never re-throttles
                        if ph % 2 == 1:
                            if jidx < len(next_jobs):
                                next_jobs[jidx]()
                                jidx += 1
                            elif ph <= 11:
                                dps = qp.tile([128, 512], f32, tag="ps")
                                for _d in range(3):
                                    nc.tensor.matmul(
                                        dps[:],
                                        lhsT=wqk_p[0][:, 0, 0:128],
                                        rhs=wqk_p[0][:, :2, :].rearrange(
                                            "p a b -> p (a b)"
                                        ),
                                        start=True,
                                        stop=True,
                                    )
                        # one deferred normalize multiply per phase, once its
                        # recip-broadcast chain has had ~4 phases to land
                        if ph >= 4:
                            _apply_norms(limit=1)
                        # S^T for step ph (64-row tiled, concurrent heads)
                        if ph < len(steps):
                            ic, jt = steps[ph]
                            i0 = 512 * ic
                            ebt = eb_tiles[(pair, ic)]
                            st2 = st_psum.tile(
                                [128, 1024], f32, tag="st",
                                name=f"st_{pair}_{ic}_{jt}",
                            )
                            for hh, ho in ((0, 0), (1, 64)):
                                nc.tensor.matmul(
                                    st2[:, 512 * hh : 512 * hh + 512],
                                    lhsT=kt[
                                        ho : ho + 64, pair,
                                        128 * jt : 128 * jt + 128,
                                    ],
                                    rhs=qt[ho : ho + 64, pair, i0 : i0 + 512],
                                    start=True,
                                    stop=True,
                                )
                            e2 = e_pool.tile([128, 1024], mmdt, tag="e")
                            nc.scalar.activation(e2[:], st2[:], Exp)
                            p2 = p_pool.tile([128, 1024], mmdt, tag="p")
                            nc.vector.tensor_tensor(
                                p2[:],
                                e2[:],
                                ebt[:, jt // 2, 1024 * (jt % 2) :
                                    1024 * (jt % 2) + 1024],
                                mult,
                            )
                            pts[(ic, jt)] = p2
                    for ic in range(IC):
                        eb_tiles.pop((pair, ic))
                    while jidx < len(next_jobs):
                        next_jobs[jidx]()
                        jidx += 1
                    if pair == HP - 1:
                        _apply_norms()

            # ---------------- output projection (transposed) ----------------
            with (
                tc.tile_pool(name="proj_psum", bufs=8, space="PSUM") as proj_psum,
                tc.tile_pool(name="o_pool", bufs=4) as o_pool,
            ):
                groups = [
                    (ft, t0)
                    for ft in range(KO)
                    for t0 in range(0, N, 512)
                ]
                LAG = min(7, len(groups) - 1) if KO > 1 else 0
                psums = {}
                for g in range(len(groups) + LAG):
                    if g < len(groups):
                        ft, t0 = groups[g]
                        ps = proj_psum.tile(
                            [128, 512], f32, tag="ps", name=f"pps_{g}"
                        )
                        for ko in range(KO - 1):
                            nc.tensor.matmul(
                                ps[:],
                                lhsT=wp[:, ko, 128 * ft : 128 * ft + 128],
                                rhs=ou[:, ko, t0 : t0 + 512],
                                start=(ko == 0),
                                stop=False,
                            )
                        psums[g] = ps
                    if g >= LAG:
                        gd = g - LAG
                        ft, t0 = groups[gd]
                        ps = psums.pop(gd)
                        nc.tensor.matmul(
                            ps[:],
                            lhsT=wp[:, KO - 1, 128 * ft : 128 * ft + 128],
                            rhs=ou[:, KO - 1, t0 : t0 + 512],
                            start=(KO == 1),
                            stop=True,
                        )
                        if gd % 2 == 0:
                            ot2 = o_pool.tile(
                                [128, 1024], mmdt, tag="ot", name=f"ot_{gd}"
                            )
                            nc.vector.tensor_copy(ot2[:, 0:512], ps[:])
                        else:
                            nc.scalar.copy(ot2[:, 512:1024], ps[:])
                            # one double-width DMA per ft row; alternate the
                            # issue queue so the tail issues in parallel
                            eng = nc.sync if ft % 2 == 0 else nc.gpsimd
                            eng.dma_start(
                                out_d[128 * ft : 128 * ft + 128, :], ot2[:]
                            )

    nc.finalize()
    return nc


def _host_pack(x, w_qkv, w_proj, b_proj, bias_table, key_padding_mask,
               N=N_TOK, H=NUM_HEADS, mmdt_name="bfloat16"):
    """Host-side layout: per-core input dicts (core i <- batch i)."""
    np_mmdt = ml_dtypes.bfloat16 if mmdt_name == "bfloat16" else np.float32
    C = H * HD
    HP = H // 2
    IC = N // 512

    x = np.asarray(x, np.float32)
    mask = np.asarray(key_padding_mask).astype(bool)
    Bb = x.shape[0]

    valid = [np.where(mask[b])[0] for b in range(Bb)]
    nv_max = max(1, max(len(v) for v in valid))
    NVT = (nv_max + 127) // 128
    NV = NVT * 128
    NJG = (NVT + 1) // 2

    w_qkv = np.asarray(w_qkv, np.float32)
    wqkT = np.ascontiguousarray(w_qkv[: 2 * C].T).astype(np.float32)
    wqkT[:, :C] *= SCALE                      # fold softmax scale into W_q
    blocks = []
    for p in range(HP):
        blocks.append(wqkT[:, 128 * p : 128 * p + 128])
        blocks.append(wqkT[:, C + 128 * p : C + 128 * p + 128])
    wqk = np.ascontiguousarray(np.concatenate(blocks, axis=1)).astype(np_mmdt)
    wv = np.ascontiguousarray(w_qkv[2 * C :].T).astype(np_mmdt)
    wp = np.ascontiguousarray(np.asarray(w_proj, np.float32).T).astype(np_mmdt)
    bp = np.asarray(b_proj, np.float32)

    etab = np.exp(np.asarray(bias_table, np.float32))   # [2N-1, H]
    iota = np.arange(N)

    in_maps = []
    for b in range(Bb):
        v = valid[b]
        nv = len(v)
        xT = np.ascontiguousarray(x[b].T).astype(np_mmdt)
        xk = np.zeros((NV, C), np.float32)
        xk[:nv] = x[b][v]
        xkT = np.ascontiguousarray(xk.T).astype(np_mmdt)
        idx = np.zeros((NV, N), np.int32)
        idx[:nv] = v[:, None] - iota[None, :] + N - 1
        eb = etab[idx, :]                     # [NV, N, H]
        eb[nv:] = 0.0
        # pad jt tiles to full groups of 2
        ebp = np.zeros((NJG * 256, N, H), np.float32)
        ebp[:NV] = eb
        # [jtg, jt01, p, ic, x, h]
        ebp = ebp.reshape(NJG, 2, 128, IC, 512, H)
        # -> [h, ic, jtg, p, jt01, x]
        ebp = ebp.transpose(5, 3, 0, 2, 1, 4)
        # -> [pair, h01, ic, jtg, p, jt01, x]
        ebp = ebp.reshape(HP, 2, IC, NJG, 128, 2, 512)
        # -> [pair, ic, jtg, p, jt01, h01, x]
        ebp = ebp.transpose(0, 2, 3, 4, 5, 1, 6)
        ebias = np.ascontiguousarray(
            ebp.reshape(HP * IC * NJG * 128, 2 * 2 * 512)
        ).astype(np_mmdt)
        in_maps.append({
            "xT": xT, "xkT": xkT, "wqk": wqk, "wv": wv, "wp": wp,
            "ebias": ebias,
        })
    return in_maps, NVT, bp


def _run(x, w_qkv, w_proj, b_proj, bias_table, key_padding_mask, trace=False):
    from concourse.bass_utils import run_bass_kernel_spmd

    in_maps, NVT, bp = _host_pack(
        x, w_qkv, w_proj, b_proj, bias_table, key_padding_mask
    )
    key = ("v4", N_TOK, NUM_HEADS, NVT)
    if key not in _BUILD_CACHE:
        _BUILD_CACHE[key] = _build_nc(NVT=NVT)
    nc = _BUILD_CACHE[key]
    res = run_bass_kernel_spmd(nc, in_maps, core_ids=list(range(B)), trace=trace)
    out = np.stack(
        [np.asarray(res.results[i]["out"]).T for i in range(B)]
    )
    return out.astype(np.float32) + bp[None, None, :], res


def kernel(x, w_qkv, w_proj, b_proj, bias_table, key_padding_mask):
    # The Bass/tile build is sensitive to Python hash randomization: with an
    # unseeded interpreter the emitted schedule is bimodal (~162-168us good
    # mode vs ~185-195us bad mode on identical source). Re-invoke ourselves
    # in a PYTHONHASHSEED=0 subprocess for the deterministic good mode; fall
    # back to an in-process run on any failure (e.g. device already held).
    import os
    import sys
    import subprocess
    import tempfile

    if os.environ.get("PYTHONHASHSEED") != "0":
        try:
            with tempfile.TemporaryDirectory() as td:
                inp = os.path.join(td, "in.npz")
                outp = os.path.join(td, "out.npy")
                np.savez(
                    inp, x=np.asarray(x), w_qkv=np.asarray(w_qkv),
                    w_proj=np.asarray(w_proj), b_proj=np.asarray(b_proj),
                    bias_table=np.asarray(bias_table),
                    key_padding_mask=np.asarray(key_padding_mask),
                )
                env = dict(os.environ, PYTHONHASHSEED="0")
                r = subprocess.run(
                    [sys.executable, os.path.abspath(__file__), inp, outp],
                    env=env, timeout=1500,
                )
                if r.returncode == 0 and os.path.exists(outp):
                    return np.load(outp)
        except Exception:
            pass
    out, _ = _run(x, w_qkv, w_proj, b_proj, bias_table, key_padding_mask)
    return out


if __name__ == "__main__":
    import sys as _sys
    _inp, _outp = _sys.argv[1], _sys.argv[2]
    _d = np.load(_inp)
    _out, _ = _run(**{k: _d[k] for k in _d.files})
    np.save(_outp, _out)


# revision 44
# speedup vs baseline: 1.1964x; 1.0281x over previous
"""Multi-head attention (B=8, N=1024, C=768, H=12) on 8 TRN2 NeuronCores.

Sharding: data-parallel over batch - core i computes batch element i fully.
Weights / bias tables are replicated. No collectives.

Final design (all matmuls bf16, f32 PSUM accumulation):
  * Key compaction: host gathers valid key rows; padded rows carry ebias=0
    so they vanish from numerator and denominator.
  * S^T via 64-row PE tiling: the two heads of a pair sit on partition
    halves of qt/kt; their 64-contraction matmuls are issued back-to-back
    at tile positions (0,0)/(64,0) into adjacent PSUM banks and execute
    concurrently (~2x on the score matmuls).
  * Attention steps flatten (query-chunk ic x key-tile jt) per head pair,
    with PV lagged 3 steps behind S^T so exp (scalar) -> P=e*ebias (DVE)
    -> PV (PE) pipelines across the ic boundary; next pair's Q/K
    projection chains interleave to keep the PE dense (HAM stays warm).
  * PSUM: qk-proj(2) + st(2x2) + pv(2) = 8 banks.
  * PV lhsT = [V | ones]: softmax denominator lands in PSUM row 64 free;
    one row copy per (pair, ic) (alternating scalar/DVE) extracts it
    FIRST in the evacuation (it heads the longest recip chain); den DMAs
    issue from the GpSimd queue, recip broadcasts from sync, and the ou
    normalization multiplies are deferred+spread into the next pair's
    phases so no engine head-of-line blocks on the recip chain.
  * Output projection transposed (out^T[f, tok]); written bf16; the host
    casts back and adds b_proj.
  * Startup: wqk is split into per-pair tiles and xT into per-chunk tiles
    (whole-tile deps) so K0/Q0 unblock after ~2.5MB of DMA instead of
    ~4.5MB; priority order xkT, wv, wqk_p0, xT_c0, ebias(0,0), xT_c1,
    wqk_p1.., ebias(0,1); wp late. Compute: warm-up, V-proj, K0/Q0,
    pair loop.
"""

import numpy as np
import ml_dtypes

DIM = 768
NUM_HEADS = 12
HD = 64
N_TOK = 1024
B = 8
SCALE = HD ** -0.5

_BUILD_CACHE = {}


def _build_nc(N=N_TOK, H=NUM_HEADS, NVT=None, mmdt_name="bfloat16"):
    import concourse.bass as bass
    import concourse.mybir as mybir
    import concourse.tile as tile
    from concourse import bacc

    # Pin every activation to the one table set containing exp/ln/copy.
    if not getattr(bacc, "_act_tables_pinned", False):
        _orig_gat = bacc.get_activation_tables

        def _pinned_gat(arch):
            tabs = _orig_gat(arch)
            want = None
            for name, funcs in tabs.items():
                fn = {f.name.lower() for f in funcs}
                if "exp" in fn and "ln" in fn and "copy" in fn:
                    want = name
                    break
            if want is None:
                return tabs
            return {
                name: (funcs if name == want else set())
                for name, funcs in tabs.items()
            }

        bacc.get_activation_tables = _pinned_gat
        bacc._act_tables_pinned = True

    f32 = mybir.dt.float32
    mmdt = getattr(mybir.dt, mmdt_name)
    Exp = mybir.ActivationFunctionType.Exp
    Ln = mybir.ActivationFunctionType.Ln
    mult = mybir.AluOpType.mult
    add = mybir.AluOpType.add

    C = H * HD
    if NVT is None:
        NVT = N // 128
    NV = NVT * 128
    KO = C // 128
    HP = H // 2
    IC = N // 512
    NJG = (NVT + 1) // 2            # jt groups of 2
    kchunks = [(k0, min(512, NV - k0)) for k0 in range(0, NV, 512)]
    fchunks = [(f0, min(512, C - f0)) for f0 in range(0, C, 512)]

    nc = bacc.Bacc(None)
    xT_d = nc.declare_dram_parameter("xT", [C, N], mmdt, isOutput=False)
    xkT_d = nc.declare_dram_parameter("xkT", [C, NV], mmdt, isOutput=False)
    # pair-major columns: [Q_p0 | K_p0 | Q_p1 | K_p1 | ...] each 128 wide
    wqk_d = nc.declare_dram_parameter("wqk", [C, 2 * C], mmdt, isOutput=False)
    wv_d = nc.declare_dram_parameter("wv", [C, C], mmdt, isOutput=False)
    wp_d = nc.declare_dram_parameter("wp", [C, C], mmdt, isOutput=False)
    # rows (pair, ic, jtg, p); cols (jt01, h01, x)
    eb_d = nc.declare_dram_parameter(
        "ebias", [HP * IC * NJG * 128, 2 * 2 * 512], mmdt, isOutput=False
    )
    out_d = nc.declare_dram_parameter("out", [C, N], mmdt, isOutput=True)

    with tile.TileContext(nc) as tc:
        with (
            tc.tile_pool(name="singles", bufs=1) as singles,
            tc.tile_pool(name="dram", bufs=1, space="DRAM") as drampool,
        ):
            # ---- input loads in priority order ----
            KH = (KO + 1) // 2
            xkT_r = xkT_d.rearrange("(ko p) n -> p ko n", p=128)
            xka = singles.tile([128, KH, NV], mmdt)
            nc.sync.dma_start(xka[:], xkT_r[:, :KH])
            xkb = None
            if KH < KO:
                xkb = singles.tile([128, KO - KH, NV], mmdt)
                nc.sync.dma_start(xkb[:], xkT_r[:, KH:])
            wv = singles.tile([128, KO, C], mmdt)
            nc.sync.dma_start(
                wv[:], wv_d.rearrange("(ko p) m -> p ko m", p=128)
            )
            wqk_r = wqk_d.rearrange("(ko p) m -> p ko m", p=128)
            wqk_p = [
                singles.tile([128, KO, 256], mmdt, name=f"wqk_p{i}")
                for i in range(HP)
            ]
            nc.sync.dma_start(wqk_p[0][:], wqk_r[:, :, 0:256])
            xT_r = xT_d.rearrange("(ko p) n -> p ko n", p=128)
            xT_c = [
                singles.tile([128, KO, 512], mmdt, name=f"xT_c{i}")
                for i in range(IC)
            ]
            nc.sync.dma_start(xT_c[0][:], xT_r[:, :, 0:512])

            def xkTs(ko):
                return xka[:, ko] if ko < KH else xkb[:, ko - KH]

            wp = singles.tile([128, KO, C], mmdt)

            vsb = singles.tile([128, NVT, H, HD + 1], mmdt)
            qt = singles.tile([128, HP, N], mmdt)
            kt = singles.tile([128, HP, NV], mmdt)
            ou = singles.tile([128, KO, N], mmdt)      # unnormalized O^T
            rb = singles.tile([128, HP, N], mmdt)      # broadcast recips
            den = singles.tile([128, N], f32)
            rden = singles.tile([128, N], f32)
            rdb = singles.tile([128, N], mmdt)
            drow = singles.tile([128, 4096], f32)      # den staging (row 64)
            rscratch = drampool.tile([H, N], mmdt)

            # ones column for the denominator (pad rows die via P=0)
            nc.vector.memset(vsb[:, :, :, HD : HD + 1], 1.0)

            eb_r = eb_d.rearrange(
                "(pr ic jtg p) x -> p pr ic jtg x", p=128, ic=IC, jtg=NJG
            )

            with (
                tc.tile_pool(name="qkv_psum", bufs=2, space="PSUM") as qp,
                tc.tile_pool(name="eb_pool", bufs=4) as eb_pool,
                tc.tile_pool(name="st_psum", bufs=2, space="PSUM") as st_psum,
                tc.tile_pool(name="pv_psum", bufs=1, space="PSUM") as pv_psum,
                tc.tile_pool(name="e_pool", bufs=3) as e_pool,
                tc.tile_pool(name="p_pool", bufs=6) as p_pool,
            ):
                if HP >= 5:
                    NORM_GROUPS = {
                        2: (0, 0, 6),
                        HP - 2: (6, 32, 2 * HP - 8),
                        HP - 1: (2 * HP - 2, 64, 2),
                    }
                else:
                    NORM_GROUPS = {HP - 1: (0, 0, H)}
                DEN_ROW = {}
                for _g0, _r0, _ng in NORM_GROUPS.values():
                    for _h in range(_g0, _g0 + _ng):
                        DEN_ROW[_h] = _r0 + _h - _g0

                pending_norms = []

                def _normalize_group(g0, r0, ng, ic):
                    i0 = 512 * ic
                    nc.scalar.activation(
                        rden[r0 : r0 + ng, i0 : i0 + 512],
                        den[r0 : r0 + ng, i0 : i0 + 512],
                        Ln,
                    )
                    nc.scalar.activation(
                        rdb[r0 : r0 + ng, i0 : i0 + 512],
                        rden[r0 : r0 + ng, i0 : i0 + 512],
                        Exp,
                        scale=-1.0,
                    )
                    nc.sync.dma_start(
                        rscratch[g0 : g0 + ng, i0 : i0 + 512],
                        rdb[r0 : r0 + ng, i0 : i0 + 512],
                    )
                    for h in range(g0, g0 + ng):
                        ho = 64 * (h % 2)
                        nc.sync.dma_start(
                            rb[ho : ho + 64, h // 2, i0 : i0 + 512],
                            bass.AP(
                                tensor=rscratch.tensor,
                                offset=rscratch[h, i0].offset,
                                ap=[[0, 64], [1, 512]],
                            ),
                        )
                    # the ou multiplies are deferred to the next pair (or
                    # the tail) so the DVE never head-of-line blocks on the
                    # recip-broadcast DMA chain, and GpSimd (which shares the
                    # DVE SBUF port) never runs tensor ops during hot compute
                    pending_norms.append((g0, ng, ic))

                norm_tts = []

                def _apply_norms(limit=None):
                    while pending_norms:
                        g0, ng, ic = pending_norms.pop(0)
                        i0 = 512 * ic
                        for sl in range(g0 // 2, (g0 + ng) // 2):
                            norm_tts.append((sl, i0))
                    n = 0
                    while norm_tts and (limit is None or n < limit):
                        sl, i0 = norm_tts.pop(0)
                        nc.vector.tensor_tensor(
                            ou[:, sl, i0 : i0 + 512],
                            ou[:, sl, i0 : i0 + 512],
                            rb[:, sl, i0 : i0 + 512],
                            mult,
                        )
                        n += 1

                # PE warm-up while input DMAs stream
                warm = e_pool.tile([128, 512], mmdt, tag="e", name="warm_in")
                nc.vector.memset(warm[:], 0.0)
                # trigger the lazy ACT_TABLE_LOAD (~2.7us) now, not at the
                # first V-projection evacuation
                nc.vector.memset(drow[0:1, 0:8], 1.0)
                nc.scalar.activation(rden[0:1, 0:8], drow[0:1, 0:8], Exp)
                wps = qp.tile([128, 512], f32, tag="ps", name="warm_ps")
                for _w in range(24):
                    nc.tensor.matmul(
                        wps[:],
                        lhsT=warm[:, :128],
                        rhs=warm[:],
                        start=True,
                        stop=True,
                    )

                # ---------- Q/K projection helper (K first: xkT+wqk arrive
                # ---------- before xT) ----------
                def qk_chunks(pair):
                    jobs = []
                    for mt, dst, chunks in (
                        (2 * pair + 1, kt, kchunks),
                        (2 * pair, qt, [(i0, 512) for i0 in range(0, N, 512)]),
                    ):
                        for i0, il in chunks:
                            def run(mt=mt, dst=dst, i0=i0, il=il, pair=pair):
                                ps = qp.tile([128, 512], f32, tag="ps")
                                co = 128 * (mt % 2)
                                for ko in range(KO):
                                    rhs = (
                                        xT_c[i0 // 512][:, ko, :il]
                                        if mt % 2 == 0
                                        else xkTs(ko)[:, i0 : i0 + il]
                                    )
                                    nc.tensor.matmul(
                                        ps[:, :il],
                                        lhsT=wqk_p[pair][:, ko, co : co + 128],
                                        rhs=rhs,
                                        start=(ko == 0),
                                        stop=(ko == KO - 1),
                                    )
                                nc.vector.tensor_copy(
                                    dst[:, pair, i0 : i0 + il], ps[:, :il]
                                )
                            jobs.append(run)
                    return jobs

                eb_tiles = {}

                def load_eb(pair, ic):
                    ebt = eb_pool.tile(
                        [128, NJG, 2048], mmdt, tag="eb",
                        name=f"eb_{pair}_{ic}",
                    )
                    nc.sync.dma_start(ebt[:], eb_r[:, pair, ic])
                    eb_tiles[(pair, ic)] = ebt

                # ---------- V projection (evacuated on scalar) ----------
                for jt in range(NVT):
                    for f0, fl in fchunks:
                        ps = qp.tile([128, 512], f32, tag="ps")
                        for ko in range(KO):
                            nc.tensor.matmul(
                                ps[:, :fl],
                                lhsT=xkTs(ko)[:, 128 * jt : 128 * jt + 128],
                                rhs=wv[:, ko, f0 : f0 + fl],
                                start=(ko == 0),
                                stop=(ko == KO - 1),
                            )
                        h0, nh = f0 // HD, fl // HD
                        nc.scalar.copy(
                            vsb[:, jt, h0 : h0 + nh, 0:HD],
                            ps[:, :fl].rearrange("p (h d) -> p h d", d=HD),
                        )

                load_eb(0, 0)
                nc.sync.dma_start(xT_c[1][:], xT_r[:, :, 512:N])
                for p_ in range(1, HP):
                    nc.sync.dma_start(
                        wqk_p[p_][:], wqk_r[:, :, 256 * p_ : 256 * p_ + 256]
                    )
                # pair 0's K+Q upfront (emitted after every input dma_start
                # above so writer-before-reader program order holds)
                for job in qk_chunks(0):
                    job()
                load_eb(0, 1)

                # late weight load (projection only)
                nc.sync.dma_start(
                    wp[:], wp_d.rearrange("(ko p) m -> p ko m", p=128)
                )

                # ---------- attention pair loop ----------
                # steps flatten (ic, jt); S^T for step k, PV for step k-2,
                # so the drain of ic0 overlaps the warm-up of ic1.
                LAGS = 4
                for pair in range(HP):
                    if pair + 1 < HP:
                        load_eb(pair + 1, 0)
                        load_eb(pair + 1, 1)
                    next_jobs = qk_chunks(pair + 1) if pair + 1 < HP else []
                    jidx = 0
                    h0, h1 = 2 * pair, 2 * pair + 1
                    steps = [(ic, jt) for ic in range(IC) for jt in range(NVT)]
                    pv_tiles = {}
                    pts = {}

                    def evac(ic, pv2, pair=pair, h0=h0, h1=h1):
                        i0 = 512 * ic
                        slot = 1024 * ((pair * IC + ic) % 4)
                        if (pair * IC + ic) % 2 == 0:
                            nc.scalar.copy(
                                drow[64:65, slot : slot + 1024],
                                pv2[HD : HD + 1, :],
                            )
                        else:
                            nc.vector.tensor_copy(
                                drow[64:65, slot : slot + 1024],
                                pv2[HD : HD + 1, :],
                            )
                        r0, r1 = DEN_ROW[h0], DEN_ROW[h1]
                        nc.gpsimd.dma_start(
                            den[r0 : r0 + 1, i0 : i0 + 512],
                            drow[64:65, slot : slot + 512],
                        )
                        nc.gpsimd.dma_start(
                            den[r1 : r1 + 1, i0 : i0 + 512],
                            drow[64:65, slot + 512 : slot + 1024],
                        )
                        if pair in NORM_GROUPS:
                            g0, r0g, ng = NORM_GROUPS[pair]
                            _normalize_group(g0, r0g, ng, ic)
                        nc.vector.tensor_copy(
                            ou[0:64, pair, i0 : i0 + 512], pv2[0:HD, 0:512]
                        )
                        nc.vector.tensor_copy(
                            ou[64:128, pair, i0 : i0 + 512],
                            pv2[0:HD, 512:1024],
                        )

                    for ph in range(len(steps) + LAGS):
                        # PV for step ph-LAGS (full 128 mode)
                        if ph >= LAGS and ph - LAGS < len(steps):
                            ic, jd = steps[ph - LAGS]
                            if jd == 0:
                                pv_tiles[ic] = pv_psum.tile(
                                    [128, 1024], f32, tag="pv",
                                    name=f"pv_{pair}_{ic}",
                                )
                            pv2 = pv_tiles[ic]
                            pd = pts.pop((ic, jd))
                            for hh in (0, 1):
                                nc.tensor.matmul(
                                    pv2[: HD + 1, 512 * hh : 512 * hh + 512],
                                    lhsT=vsb[:, jd, h0 + hh, :],
                                    rhs=pd[:, 512 * hh : 512 * hh + 512],
                                    start=(jd == 0),
                                    stop=(jd == NVT - 1),
                                )
                            if jd == NVT - 1:
                                evac(ic, pv_tiles.pop(ic))
                        # one Q/K chain of the next pair; once a pair's
                        # chains run out (and for the last pair, always),
                        # feed dummy matmuls so HAM never re-throttles
                        if ph % 2 == 1:
                            if jidx < len(next_jobs):
                                next_jobs[jidx]()
                                jidx += 1
                            elif ph <= 11:
                                dps = qp.tile([128, 512], f32, tag="ps")
                                for _d in range(3):
                                    nc.tensor.matmul(
                                        dps[:],
                                        lhsT=wqk_p[0][:, 0, 0:128],
                                        rhs=wqk_p[0][:, :2, :].rearrange(
                                            "p a b -> p (a b)"
                                        ),
                                        start=True,
                                        stop=True,
                                    )
                        # one deferred normalize multiply per phase, once its
                        # recip-broadcast chain has had ~4 phases to land
                        if ph >= 4:
                            _apply_norms(limit=1)
                        # S^T for step ph (64-row tiled, concurrent heads)
                        if ph < len(steps):
                            ic, jt = steps[ph]
                            i0 = 512 * ic
                            ebt = eb_tiles[(pair, ic)]
                            st2 = st_psum.tile(
                                [128, 1024], f32, tag="st",
                                name=f"st_{pair}_{ic}_{jt}",
                            )
                            for hh, ho in ((0, 0), (1, 64)):
                                nc.tensor.matmul(
                                    st2[:, 512 * hh : 512 * hh + 512],
                                    lhsT=kt[
                                        ho : ho + 64, pair,
                                        128 * jt : 128 * jt + 128,
                                    ],
                                    rhs=qt[ho : ho + 64, pair, i0 : i0 + 512],
                                    start=True,
                                    stop=True,
                                )
                            e2 = e_pool.tile([128, 1024], mmdt, tag="e")
                            nc.scalar.activation(e2[:], st2[:], Exp)
                            p2 = p_pool.tile([128, 1024], mmdt, tag="p")
                            nc.vector.tensor_tensor(
                                p2[:],
                                e2[:],
                                ebt[:, jt // 2, 1024 * (jt % 2) :
                                    1024 * (jt % 2) + 1024],
                                mult,
                            )
                            pts[(ic, jt)] = p2
                    for ic in range(IC):
                        eb_tiles.pop((pair, ic))
                    while jidx < len(next_jobs):
                        next_jobs[jidx]()
                        jidx += 1
                    if pair == HP - 1:
                        _apply_norms()

            # ---------------- output projection (transposed) ----------------
            with (
                tc.tile_pool(name="proj_psum", bufs=8, space="PSUM") as proj_psum,
                tc.tile_pool(name="o_pool", bufs=4) as o_pool,
            ):
                groups = [
                    (ft, t0)
                    for ft in range(KO)
                    for t0 in range(0, N, 512)
                ]
                LAG = min(7, len(groups) - 1) if KO > 1 else 0
                psums = {}
                for g in range(len(groups) + LAG):
                    if g < len(groups):
                        ft, t0 = groups[g]
                        ps = proj_psum.tile(
                            [128, 512], f32, tag="ps", name=f"pps_{g}"
                        )
                        for ko in range(KO - 1):
                            nc.tensor.matmul(
                                ps[:],
                                lhsT=wp[:, ko, 128 * ft : 128 * ft + 128],
                                rhs=ou[:, ko, t0 : t0 + 512],
                                start=(ko == 0),
                                stop=False,
                            )
                        psums[g] = ps
                    if g >= LAG:
                        gd = g - LAG
                        ft, t0 = groups[gd]
                        ps = psums.pop(gd)
                        nc.tensor.matmul(
                            ps[:],
                            lhsT=wp[:, KO - 1, 128 * ft : 128 * ft + 128],
                            rhs=ou[:, KO - 1, t0 : t0 + 512],
                            start=(KO == 1),
                            stop=True,
                        )
                        if gd % 2 == 0:
                            ot2 = o_pool.tile(
                                [128, 1024], mmdt, tag="ot", name=f"ot_{gd}"
                            )
                            nc.vector.tensor_copy(ot2[:, 0:512], ps[:])
                        else:
                            nc.scalar.copy(ot2[:, 512:1024], ps[:])
                            # one double-width DMA per ft row; alternate the
                            # issue queue so the tail issues in parallel
                            eng = nc.sync if ft % 2 == 0 else nc.gpsimd
                            eng.dma_start(
                                out_d[128 * ft : 128 * ft + 128, :], ot2[:]
                            )

    nc.finalize()
    return nc


def _host_pack(x, w_qkv, w_proj, b_proj, bias_table, key_padding_mask,
               N=N_TOK, H=NUM_HEADS, mmdt_name="bfloat16"):
    """Host-side layout: per-core input dicts (core i <- batch i)."""
    np_mmdt = ml_dtypes.bfloat16 if mmdt_name == "bfloat16" else np.float32
    C = H * HD
    HP = H // 2
    IC = N // 512

    x = np.asarray(x, np.float32)
    mask = np.asarray(key_padding_mask).astype(bool)
    Bb = x.shape[0]

    valid = [np.where(mask[b])[0] for b in range(Bb)]
    nv_max = max(1, max(len(v) for v in valid))
    NVT = (nv_max + 127) // 128
    NV = NVT * 128
    NJG = (NVT + 1) // 2

    w_qkv = np.asarray(w_qkv, np.float32)
    wqkT = np.ascontiguousarray(w_qkv[: 2 * C].T).astype(np.float32)
    wqkT[:, :C] *= SCALE                      # fold softmax scale into W_q
    blocks = []
    for p in range(HP):
        blocks.append(wqkT[:, 128 * p : 128 * p + 128])
        blocks.append(wqkT[:, C + 128 * p : C + 128 * p + 128])
    wqk = np.ascontiguousarray(np.concatenate(blocks, axis=1)).astype(np_mmdt)
    wv = np.ascontiguousarray(w_qkv[2 * C :].T).astype(np_mmdt)
    wp = np.ascontiguousarray(np.asarray(w_proj, np.float32).T).astype(np_mmdt)
    bp = np.asarray(b_proj, np.float32)

    etab = np.exp(np.asarray(bias_table, np.float32))   # [2N-1, H]
    iota = np.arange(N)

    in_maps = []
    for b in range(Bb):
        v = valid[b]
        nv = len(v)
        xT = np.ascontiguousarray(x[b].T).astype(np_mmdt)
        xk = np.zeros((NV, C), np.float32)
        xk[:nv] = x[b][v]
        xkT = np.ascontiguousarray(xk.T).astype(np_mmdt)
        idx = np.zeros((NV, N), np.int32)
        idx[:nv] = v[:, None] - iota[None, :] + N - 1
        eb = etab[idx, :]                     # [NV, N, H]
        eb[nv:] = 0.0
        # pad jt tiles to full groups of 2
        ebp = np.zeros((NJG * 256, N, H), np.float32)
        ebp[:NV] = eb
        # [jtg, jt01, p, ic, x, h]
        ebp = ebp.reshape(NJG, 2, 128, IC, 512, H)
        # -> [h, ic, jtg, p, jt01, x]
        ebp = ebp.transpose(5, 3, 0, 2, 1, 4)
        # -> [pair, h01, ic, jtg, p, jt01, x]
        ebp = ebp.reshape(HP, 2, IC, NJG, 128, 2, 512)
        # -> [pair, ic, jtg, p, jt01, h01, x]
        ebp = ebp.transpose(0, 2, 3, 4, 5, 1, 6)
        ebias = np.ascontiguousarray(
            ebp.reshape(HP * IC * NJG * 128, 2 * 2 * 512)
        ).astype(np_mmdt)
        in_maps.append({
            "xT": xT, "xkT": xkT, "wqk": wqk, "wv": wv, "wp": wp,
            "ebias": ebias,
        })
    return in_maps, NVT, bp


def _run(x, w_qkv, w_proj, b_proj, bias_table, key_padding_mask, trace=False):
    from concourse.bass_utils import run_bass_kernel_spmd

    in_maps, NVT, bp = _host_pack(
        x, w_qkv, w_proj, b_proj, bias_table, key_padding_mask
    )
    key = ("v4", N_TOK, NUM_HEADS, NVT)
    if key not in _BUILD_CACHE:
        _BUILD_CACHE[key] = _build_nc(NVT=NVT)
    nc = _BUILD_CACHE[key]
    res = run_bass_kernel_spmd(nc, in_maps, core_ids=list(range(B)), trace=trace)
    out = np.stack(
        [np.asarray(res.results[i]["out"]).T for i in range(B)]
    )
    return out.astype(np.float32) + bp[None, None, :], res


def kernel(x, w_qkv, w_proj, b_proj, bias_table, key_padding_mask):
    # The Bass/tile build is sensitive to Python hash randomization: with an
    # unseeded interpreter the emitted schedule is bimodal (~162-168us good
    # mode vs ~185-195us bad mode on identical source). Re-invoke ourselves
    # in a PYTHONHASHSEED=0 subprocess for the deterministic good mode; fall
    # back to an in-process run on any failure (e.g. device already held).
    import os
    import sys
    import subprocess
    import tempfile

    if os.environ.get("PYTHONHASHSEED") != "0":
        try:
            with tempfile.TemporaryDirectory() as td:
                inp = os.path.join(td, "in.npz")
                outp = os.path.join(td, "out.npy")
                np.savez(
                    inp, x=np.asarray(x), w_qkv=np.asarray(w_qkv),
                    w_proj=np.asarray(w_proj), b_proj=np.asarray(b_proj),
                    bias_table=np.asarray(bias_table),
                    key_padding_mask=np.asarray(key_padding_mask),
                )
                env = dict(os.environ, PYTHONHASHSEED="0")
                r = subprocess.run(
                    [sys.executable, os.path.abspath(__file__), inp, outp],
                    env=env, timeout=1500,
                )
                if r.returncode == 0 and os.path.exists(outp):
                    return np.load(outp)
        except Exception:
            pass
    out, _ = _run(x, w_qkv, w_proj, b_proj, bias_table, key_padding_mask)
    return out


if __name__ == "__main__":
    import sys as _sys
    _inp, _outp = _sys.argv[1], _sys.argv[2]
    _d = np.load(_inp)
    _out, _ = _run(**{k: _d[k] for k in _d.files})
    np.save(_outp, _out)
